# revision 1
# baseline (speedup 1.0000x reference)
"""Trainium2 Bass kernel for nn_DecoupleModel (GNN message passing, 3 MP layers + MLP tail).

Self-contained: call kernel(**inputs) with the full (unsharded) inputs from
setup_inputs(); returns the full [N, 64] float32 output.

Strategy (8 NeuronCores, node-parallel / graph-parallel):
  - Nodes are sharded by contiguous orig-id range (12500/core, padded to 12544).
  - Per layer: feat-major W matmul (bf16) -> relu(+bias) -> PE transpose to
    node-major -> per-node 1/norm scale -> cast+DMA shard to DRAM -> AllGather
    the feature table (halo exchange) -> edge aggregation -> merge
    (1+eps)*h + Ah -> transpose back to feat-major.
  - Edge aggregation Ah[t] = sum_e h[src(e)] uses the TIE-accelerated SWDGE
    ops: dma_gather pulls per-edge source rows (int16 indices relative to a
    <=32768-row band of the table) into SBUF, and dma_scatter_add (SBUF
    parity-split destination mode) accumulates them into two SBUF
    accumulators keyed by local target id. Edges are grouped by source band
    on the host; within a band order is arbitrary (the scatter ring handles
    duplicate targets in order).
  - MLP tail computed feat-major with PSUM-fused injection branches.
  - Host reassembles the final output from the per-core shards.
"""

import numpy as np

import concourse.bass as bass
import concourse.bacc as bacc
import concourse.mybir as mybir
import concourse.tile as tile
from concourse.bass_utils import run_bass_kernel_spmd
from concourse.masks import make_identity

EPS = 2.0 ** 0.5
NCORES = 8
F32 = mybir.dt.float32
BF16 = mybir.dt.bfloat16
I16 = mybir.dt.int16
ACT_DT = BF16                      # activations / weights dtype
NP_ACT = mybir.dt.np(ACT_DT)
TBL_DT = BF16                      # feature table / gather / accumulate dtype
NP_TBL = mybir.dt.np(TBL_DT)
W32 = 32768                        # int16 index window (table band rows)
CALL = 4096                        # edges per call (single_packet=False lifts the 1024 packet limit)

LAST_RESULTS = None


# ----------------------------------------------------------------------------
# Host-side graph preprocessing
# ----------------------------------------------------------------------------

def _prep_graph(N, edge_index):
    tgt = np.asarray(edge_index[0]).astype(np.int64)
    src = np.asarray(edge_index[1]).astype(np.int64)

    n_loc = N // NCORES
    CH = -(-n_loc // 128)
    LOC = CH * 128
    NR = NCORES * LOC

    deg_src = np.bincount(src, minlength=N)
    inv_norm = (1.0 / (1.0 + EPS + deg_src.astype(np.float64))).astype(np.float32)

    # local nid = original local index; DRAM table row of orig node n:
    # core*LOC + (nid%128)*CH + nid//128  (so the shard DMA is contiguous)
    nid = np.arange(N) % n_loc
    trow = (np.arange(N) // n_loc) * LOC + (nid % 128) * CH + nid // 128

    n_bands = -(-NR // W32)
    band_of = trow // W32

    # Within each band, split edges into "rounds": round r holds each
    # target's r-th in-band edge, so every scatter call has unique targets
    # (the CCE read-modify-write pipelines and loses updates on intra-call
    # duplicate destinations).
    per_core = []  # [core][band] -> (gidx, sidx, round_ids)
    for c in range(NCORES):
        m = (tgt // n_loc) == c
        et, es = tgt[m], src[m]
        bt = band_of[es]
        lists = []
        for b in range(n_bands):
            mb_ = bt == b
            gi = (trow[es[mb_]] - b * W32).astype(np.int16)
            si = (et[mb_] % n_loc).astype(np.int16)
            o = np.argsort(si, kind="stable")
            si, gi = si[o], gi[o]
            rr = np.arange(si.size) - np.searchsorted(si, si, side="left")
            o2 = np.lexsort((si, rr))
            lists.append((gi[o2], si[o2], rr[o2]))
        per_core.append(lists)

    # shared segment structure: per (band, round), size = max over cores, x128
    seg_sizes = []  # [(band, round, size)]
    for b in range(n_bands):
        rmax = max((int(per_core[c][b][2][-1]) + 1 if per_core[c][b][2].size else 0)
                   for c in range(NCORES))
        for r in range(rmax):
            mx = max(int((per_core[c][b][2] == r).sum()) for c in range(NCORES))
            seg_sizes.append((b, r, -(-max(mx, 16) // 128) * 128))
    tot = sum(s for _, _, s in seg_sizes)
    WG = tot // 16

    TRASH = LOC  # scatter target for pad edges (trash column; duplicates ok)

    seg_off = {}
    off = 0
    for (b, r, sz) in seg_sizes:
        seg_off[(b, r)] = off
        off += sz

    gpacks, spacks = [], []
    for c in range(NCORES):
        gflat = np.zeros(tot, np.int16)
        sflat = np.full(tot, TRASH, np.int16)
        for b in range(n_bands):
            gi, si, rr = per_core[c][b]
            if not gi.size:
                continue
            starts = np.searchsorted(rr, np.arange(rr[-1] + 2))
            for r in range(int(rr[-1]) + 1):
                a, e = int(starts[r]), int(starts[r + 1])
                o = seg_off[(b, r)]
                gflat[o:o + e - a] = gi[a:e]
                sflat[o:o + e - a] = si[a:e]
        # token m -> idx tile position (m % 16, m // 16); the SWDGE tx/rx
        # Q7 cores read different partition groups -> replicate to all 128
        gpacks.append(np.tile(gflat.reshape(-1, 16).T, (8, 1)))
        spacks.append(np.tile(sflat.reshape(-1, 16).T, (8, 1)))

    # call list: (band, token_offset, n_tokens); never spans a segment
    calls = []
    for (b, r, sz) in seg_sizes:
        off = seg_off[(b, r)]
        p = 0
        while p < sz:
            n = min(CALL, sz - p)
            calls.append((b, off + p, n))
            p += n

    # per-core 1/norm swizzled [128, CH]
    norm_sw = []
    for c in range(NCORES):
        v = np.zeros((128, CH), np.float32)
        nids = np.arange(n_loc)
        v[nids % 128, nids // 128] = inv_norm[c * n_loc:(c + 1) * n_loc]
        norm_sw.append(v)

    return dict(
        n_loc=n_loc, CH=CH, LOC=LOC, NR=NR, WG=WG, n_bands=n_bands,
        gpacks=gpacks, spacks=spacks, calls=calls, norm_sw=norm_sw,
    )


# ----------------------------------------------------------------------------
# Bass kernel build
# ----------------------------------------------------------------------------

def _build_bass(D, FL, OD, CH, LOC, NR, WG, calls):
    assert D == 128
    nc = bacc.Bacc()

    xT = nc.declare_dram_parameter("xT", [D, LOC], ACT_DT, isOutput=False)
    wmp = nc.declare_dram_parameter("wmp", [D, 3 * D], ACT_DT, isOutput=False)
    bmp = nc.declare_dram_parameter("bmp", [D, 3], F32, isOutput=False)
    nrm = nc.declare_dram_parameter("nrm", [D, CH], F32, isOutput=False)
    gidx = nc.declare_dram_parameter("gidx", [128, WG], I16, isOutput=False)
    sidx = nc.declare_dram_parameter("sidx", [128, WG], I16, isOutput=False)
    MF = FL // 128
    fc0 = nc.declare_dram_parameter("fc0", [D, FL], ACT_DT, isOutput=False)
    p0 = nc.declare_dram_parameter("p0", [D, FL], ACT_DT, isOutput=False)
    fc1 = nc.declare_dram_parameter("fc1", [128, MF * FL], ACT_DT, isOutput=False)
    p1 = nc.declare_dram_parameter("p1", [D, FL], ACT_DT, isOutput=False)
    ow = nc.declare_dram_parameter("ow", [128, MF * OD], ACT_DT, isOutput=False)
    bz1 = nc.declare_dram_parameter("bz1", [128, MF], F32, isOutput=False)
    bz2 = nc.declare_dram_parameter("bz2", [128, MF], F32, isOutput=False)
    bo = nc.declare_dram_parameter("bo", [128, 1], F32, isOutput=False)
    out_t = nc.declare_dram_parameter("out_t", [OD, LOC], F32, isOutput=True)

    NT = -(-LOC // 512)
    JC = CALL // 128          # gather buffer chunks per (full) call
    GSL = CH // 2 + 1         # accumulator free slots (+1 trash)
    TROWS = -(-NR // 128) * 128 + 128

    with tile.TileContext(nc) as tc:
        with (
            tc.tile_pool(name="dram", bufs=1, space="DRAM") as dram,
            tc.tile_pool(name="big", bufs=1) as big,
            tc.tile_pool(name="wts", bufs=1) as wts,
            tc.tile_pool(name="gb", bufs=3) as gb,
            tc.tile_pool(name="work", bufs=3) as work,
            tc.tile_pool(name="psmm", bufs=3, space="PSUM") as psmm,
            tc.tile_pool(name="pstr", bufs=3, space="PSUM") as pstr,
        ):
            shard = dram.tile([LOC, D], TBL_DT)
            table = dram.tile([TROWS, D], TBL_DT)

            fm = big.tile([128, LOC], ACT_DT, tag="fm")
            nmA = big.tile([128, LOC], ACT_DT, tag="nm")
            accE = big.tile([128, GSL * 128], TBL_DT, tag="ae")
            accO = big.tile([128, GSL * 128], TBL_DT, tag="ao")

            n2 = wts.tile([128, CH], F32)
            wmm = wts.tile([128, 3 * D], ACT_DT)
            bcol = wts.tile([128, 3], F32)
            ident = wts.tile([128, 128], ACT_DT)
            fc0_s = wts.tile([128, FL], ACT_DT)
            p0_s = wts.tile([128, FL], ACT_DT)
            fc1_s = wts.tile([128, MF * FL], ACT_DT)
            p1_s = wts.tile([128, FL], ACT_DT)
            ow_s = wts.tile([128, MF * OD], ACT_DT)
            bz1_s = wts.tile([128, MF], F32)
            bz2_s = wts.tile([128, MF], F32)
            bo_s = wts.tile([128, 1], F32)

            make_identity(nc, ident[:])

            nc.sync.dma_start(out=fm[:], in_=xT[:, :])
            nc.sync.dma_start(out=n2[:], in_=nrm[:, :])
            nc.sync.dma_start(out=wmm[:], in_=wmp[:, :])
            nc.sync.dma_start(out=bcol[:], in_=bmp[:, :])
            nc.sync.dma_start(out=fc0_s[:], in_=fc0[:, :])
            nc.sync.dma_start(out=p0_s[:], in_=p0[:, :])
            nc.sync.dma_start(out=fc1_s[:], in_=fc1[:, :])
            nc.sync.dma_start(out=p1_s[:], in_=p1[:, :])
            nc.sync.dma_start(out=ow_s[:], in_=ow[:, :])
            nc.sync.dma_start(out=bz1_s[:], in_=bz1[:, :])
            nc.sync.dma_start(out=bz2_s[:], in_=bz2[:, :])
            nc.sync.dma_start(out=bo_s[:], in_=bo[:, :])

            for L in range(3):
                # h_pre = prev @ W + b, relu (feat-major, in place)
                for t in range(NT):
                    w = min(512, LOC - t * 512)
                    ps = psmm.tile([128, 512], F32, tag="mm")
                    nc.tensor.matmul(
                        out=ps[:, :w], lhsT=wmm[:, L * D:(L + 1) * D],
                        rhs=fm[:, t * 512:t * 512 + w], start=True, stop=True)
                    nc.scalar.activation(
                        out=fm[:, t * 512:t * 512 + w], in_=ps[:, :w],
                        func=mybir.ActivationFunctionType.Relu,
                        bias=bcol[:, L:L + 1], scale=1.0)
                # transpose to node-major + 1/norm scale
                for ch in range(CH):
                    pt = pstr.tile([128, 128], ACT_DT, tag="tr")
                    nc.tensor.transpose(
                        out=pt[:], in_=fm[:, ch * 128:(ch + 1) * 128],
                        identity=ident[:])
                    nc.vector.tensor_scalar(
                        out=nmA[:, ch * 128:(ch + 1) * 128], in0=pt[:],
                        scalar1=n2[:, ch:ch + 1], scalar2=None,
                        op0=mybir.AluOpType.mult)
                # publish shard and exchange (HWDGE when no dtype cast needed)
                eng = nc.sync if TBL_DT == ACT_DT else nc.gpsimd
                eng.dma_start(
                    out=shard[:].rearrange("(p x) d -> p (x d)", p=128),
                    in_=nmA[:])
                nc.gpsimd.collective_compute(
                    "AllGather", mybir.AluOpType.bypass,
                    ins=[shard[:]], outs=[table[0:NR, :]],
                    replica_groups=[list(range(NCORES))])
                # edge aggregation: gather source rows, scatter-add by target
                nc.vector.memset(accE[:], 0.0)
                nc.vector.memset(accO[:], 0.0)
                for (b, off, n) in calls:
                    gbuf = gb.tile([128, JC * 128], TBL_DT, tag="gbuf")
                    gt = work.tile([128, CALL // 16], I16, tag="gt")
                    st = work.tile([128, CALL // 16], I16, tag="st")
                    nc.sync.dma_start(out=gt[:, :n // 16],
                                      in_=gidx[:, off // 16:(off + n) // 16])
                    nc.sync.dma_start(out=st[:, :n // 16],
                                      in_=sidx[:, off // 16:(off + n) // 16])
                    nc.gpsimd.dma_gather(
                        out_ap=gbuf[:, :n].rearrange("p (j e) -> p j e", e=128),
                        in_ap=table[b * W32:min(b * W32 + W32, NR), :],
                        idxs_ap=gt[:, :n // 16],
                        num_idxs=n, num_idxs_reg=n, elem_size=128,
                        single_packet=False)
                    nc.gpsimd.dma_scatter_add(
                        out_ap=accE[:],
                        in_ap=gbuf[:, :n].rearrange("p (j e) -> p j e", e=128),
                        idxs_ap=st[:, :n // 16],
                        num_idxs=n, num_idxs_reg=n, elem_size=128,
                        sbuf_tokens_per_rank=128, parity_reg=0,
                        out_ap_other=accO[:], single_packet=False)
                # merge: nmA = (1+eps)*nmA + Ah (node-major, in place)
                for ch in range(CH):
                    acc = accE if ch % 2 == 0 else accO
                    gsl = ch // 2
                    nc.vector.scalar_tensor_tensor(
                        out=nmA[:, ch * 128:(ch + 1) * 128],
                        in0=nmA[:, ch * 128:(ch + 1) * 128],
                        scalar=float(1.0 + EPS),
                        in1=acc[:, gsl * 128:(gsl + 1) * 128],
                        op0=mybir.AluOpType.mult, op1=mybir.AluOpType.add)
                # transpose back to feat-major
                for ch in range(CH):
                    pt = pstr.tile([128, 128], ACT_DT, tag="tr")
                    nc.tensor.transpose(
                        out=pt[:], in_=nmA[:, ch * 128:(ch + 1) * 128],
                        identity=ident[:])
                    nc.scalar.activation(
                        out=fm[:, ch * 128:(ch + 1) * 128], in_=pt[:],
                        func=mybir.ActivationFunctionType.Copy, scale=1.0)

            # ---------------- MLP tail (feat-major) ----------------
            zr1 = big.tile([128, MF * 512], ACT_DT, tag="nm")
            z2 = big.tile([128, MF * 512], ACT_DT, tag="ae")
            for t in range(NT):
                w = min(512, LOC - t * 512)
                sl = slice(t * 512, t * 512 + w)
                rlH = work.tile([128, 512], ACT_DT, tag="rlH")
                nc.scalar.activation(
                    out=rlH[:, :w], in_=fm[:, sl],
                    func=mybir.ActivationFunctionType.Relu, scale=1.0)
                for m in range(MF):
                    ps = psmm.tile([128, 512], F32, tag="mm")
                    nc.tensor.matmul(
                        out=ps[:, :w], lhsT=fc0_s[:, m * 128:(m + 1) * 128],
                        rhs=rlH[:, :w], start=True, stop=False)
                    nc.tensor.matmul(
                        out=ps[:, :w], lhsT=p0_s[:, m * 128:(m + 1) * 128],
                        rhs=fm[:, sl], start=False, stop=True)
                    nc.scalar.activation(
                        out=zr1[:, m * 512:m * 512 + w], in_=ps[:, :w],
                        func=mybir.ActivationFunctionType.Relu,
                        bias=bz1_s[:, m:m + 1], scale=1.0)
                for m in range(MF):
                    ps = psmm.tile([128, 512], F32, tag="mm")
                    for k in range(MF):
                        nc.tensor.matmul(
                            out=ps[:, :w],
                            lhsT=fc1_s[:, k * FL + m * 128:k * FL + (m + 1) * 128],
                            rhs=zr1[:, k * 512:k * 512 + w],
                            start=(k == 0), stop=False)
                    nc.tensor.matmul(
                        out=ps[:, :w], lhsT=p1_s[:, m * 128:(m + 1) * 128],
                        rhs=fm[:, sl], start=False, stop=True)
                    nc.vector.tensor_scalar(
                        out=z2[:, m * 512:m * 512 + w], in0=ps[:, :w],
                        scalar1=bz2_s[:, m:m + 1], scalar2=None,
                        op0=mybir.AluOpType.add)
                pso = psmm.tile([128, 512], F32, tag="mm")
                for k in range(MF):
                    nc.tensor.matmul(
                        out=pso[:OD, :w], lhsT=ow_s[:, k * OD:(k + 1) * OD],
                        rhs=z2[:, k * 512:k * 512 + w],
                        start=(k == 0), stop=(k == MF - 1))
                osb = work.tile([OD, 512], F32, tag="osb")
                nc.vector.tensor_scalar(
                    out=osb[:, :w], in0=pso[:OD, :w], scalar1=bo_s[:OD, :],
                    scalar2=None, op0=mybir.AluOpType.add)
                nc.sync.dma_start(out=out_t[:, sl], in_=osb[:, :w])

    nc.compile()
    return nc


# ----------------------------------------------------------------------------
# Entry point
# ----------------------------------------------------------------------------

def kernel(x, edge_index, mpW0, mpb0, mpW1, mpb1, mpW2, mpb2,
           fcW0, fcb0, fcW1, fcb1, pW0, pb0, pW1, pb1, outW, outb,
           _run=None):
    global LAST_RESULTS
    x = np.asarray(x)
    N, D = x.shape
    FL = fcW0.shape[1]
    OD = outW.shape[1]
    MF = FL // 128
    g = _prep_graph(N, edge_index)
    CH, LOC, NR, WG = g["CH"], g["LOC"], g["NR"], g["WG"]

    s = np.float32(1.0 / np.sqrt(np.float32(D)))
    wmp = np.concatenate([np.asarray(w, np.float32) * s
                          for w in (mpW0, mpW1, mpW2)], axis=1).astype(NP_ACT)
    bmp = np.stack([np.asarray(b, np.float32) * s
                    for b in (mpb0, mpb1, mpb2)], axis=1)

    fc1_pack = np.asarray(fcW1, np.float32).reshape(MF, 128, FL)
    fc1_pack = fc1_pack.transpose(1, 0, 2).reshape(128, MF * FL).astype(NP_ACT)
    ow_pack = np.asarray(outW, np.float32).reshape(MF, 128, OD)
    ow_pack = ow_pack.transpose(1, 0, 2).reshape(128, MF * OD).astype(NP_ACT)
    bz1 = (np.asarray(fcb0, np.float32) + np.asarray(pb0, np.float32)).reshape(MF, 128).T.copy()
    bz2 = (np.asarray(fcb1, np.float32) + np.asarray(pb1, np.float32)).reshape(MF, 128).T.copy()
    bo = np.zeros((128, 1), np.float32)
    bo[:OD, 0] = np.asarray(outb, np.float32)

    nc = _build_bass(D, FL, OD, CH, LOC, NR, WG, g["calls"])

    n_loc = g["n_loc"]
    in_maps = []
    for c in range(NCORES):
        xt = np.zeros((D, LOC), NP_ACT)
        xt[:, :n_loc] = x[c * n_loc:(c + 1) * n_loc].T.astype(NP_ACT)
        in_maps.append(dict(
            xT=xt, wmp=wmp, bmp=bmp, nrm=g["norm_sw"][c],
            gidx=g["gpacks"][c], sidx=g["spacks"][c],
            fc0=np.asarray(fcW0, np.float32).astype(NP_ACT),
            p0=np.asarray(pW0, np.float32).astype(NP_ACT),
            fc1=fc1_pack, p1=np.asarray(pW1, np.float32).astype(NP_ACT),
            ow=ow_pack,
            bz1=bz1, bz2=bz2, bo=bo,
        ))

    if _run is None:
        res = run_bass_kernel_spmd(nc, in_maps, list(range(NCORES)), trace=False)
        LAST_RESULTS = res
        outs = [res.results[c]["out_t"] for c in range(NCORES)]
    else:
        outs = _run(nc, in_maps)

    out = np.empty((N, OD), np.float32)
    for c in range(NCORES):
        o = np.asarray(outs[c]).T  # [LOC, OD], row nid
        out[c * n_loc:(c + 1) * n_loc] = o[:n_loc]
    return out



# revision 17
# speedup vs baseline: 16.3706x; 16.3706x over previous
"""Trainium2 Bass kernel for nn_DecoupleModel (GNN message passing, 3 MP layers + MLP tail).

Self-contained: call kernel(**inputs) with the full (unsharded) inputs from
setup_inputs(); returns the full [N, 64] float32 output.

Strategy (8 NeuronCores, node-parallel / graph-parallel):
  - Nodes are sharded by contiguous orig-id range (12500/core, padded to 12544).
  - Per layer: feat-major W matmul (bf16) -> relu(+bias) -> per-node 1/norm
    scale (feat-major, via a host-broadcast column-scale matrix) -> PE
    transpose to node-major -> DMA shard to DRAM -> AllGather the feature
    table into Shared DRAM (halo exchange).
  - Edge aggregation Ah[t] = sum_e h[src(e)] is gather + PE segment-matmul:
    edges are grouped host-side by (source band, target group of 128) and
    padded per segment to a multiple of 128 (sizes shared across cores for
    SPMD). dma_gather (SWDGE) pulls per-edge source rows into SBUF tiles of
    128 edges x 128 feat; a one-hot matrix S (built on the DVE by comparing
    an iota row against each token's target offset) turns the segment-sum
    into psum_fm[feat, tgt] += g_tile^T.T @ S_tile accumulated in f32 PSUM.
    Partials merge into the (feat-major) next-layer activations with
    (1+eps)*h + Ah on the DVE, so no scatter DMA and no transpose back.
  - MLP tail computed feat-major with PSUM-fused injection branches.
  - Host reassembles the final output from the per-core shards.
"""

import os

import numpy as np

import concourse.bass as bass
import concourse.bacc as bacc
import concourse.mybir as mybir
import concourse.tile as tile
from concourse.bass_utils import run_bass_kernel_spmd
from concourse.masks import make_identity

EPS = 2.0 ** 0.5
NCORES = 8
F32 = mybir.dt.float32
BF16 = mybir.dt.bfloat16
I16 = mybir.dt.int16
ACT_DT = BF16                      # activations / weights dtype
NP_ACT = mybir.dt.np(ACT_DT)
TBL_DT = BF16                      # feature table / gather dtype
CALL = 4096                        # max gather tokens per SWDGE call
NQ = int(os.environ.get("NQ", "4"))  # SWDGE queues used (round-robin)

LAST_RESULTS = None

# ablation flags (bench only; all default off)
_SKIP_EDGE = bool(int(os.environ.get("SKIP_EDGE", "0")))
_SKIP_SCATTER = bool(int(os.environ.get("SKIP_SCATTER", "0")))
_SKIP_GATHER = bool(int(os.environ.get("SKIP_GATHER", "0")))
_SKIP_AG = bool(int(os.environ.get("SKIP_AG", "0")))
_SKIP_TAIL = bool(int(os.environ.get("SKIP_TAIL", "0")))
_N_LAYERS = int(os.environ.get("N_LAYERS", "3"))


# ----------------------------------------------------------------------------
# Host-side graph preprocessing
# ----------------------------------------------------------------------------

def _prep_graph(N, edge_index):
    tgt = np.asarray(edge_index[0]).astype(np.int64)
    src = np.asarray(edge_index[1]).astype(np.int64)

    n_loc = N // NCORES
    CH = -(-n_loc // 128)
    LOC = CH * 128
    NR = NCORES * LOC
    BAND = 2 * LOC                 # int16-indexable gather window (<= 32768)
    NB = -(-NR // BAND)
    NG = CH                        # target groups of 128 (psum/merge blocks)

    deg_src = np.bincount(src, minlength=N)
    inv_norm = (1.0 / (1.0 + EPS + deg_src.astype(np.float64))).astype(np.float32)

    # table row of orig node n: core*LOC + (nid%128)*CH + nid//128
    # (so the per-chunk node-major transpose output DMAs contiguously)
    allnid = np.arange(N) % n_loc
    trow = (np.arange(N) // n_loc) * LOC + (allnid % 128) * CH + allnid // 128

    core_of = tgt // n_loc
    tnid = tgt % n_loc
    g_all = tnid // 128
    toff_all = tnid % 128
    erow = trow[src]
    b_all = erow // BAND
    gval_all = erow % BAND

    # shared segment sizes: per (band, group), max over cores, padded to 128
    keys = (core_of * NB + b_all) * NG + g_all
    cnt = np.bincount(keys, minlength=NCORES * NB * NG).reshape(NCORES, NB, NG)
    seg = (-(-cnt.max(axis=0) // 128) * 128).astype(np.int64)  # [NB, NG]

    off = np.zeros((NB, NG), np.int64)
    tot = 0
    for b in range(NB):
        for g in range(NG):
            off[b, g] = tot
            tot += int(seg[b, g])
    TOT = tot

    # call list: per band, consecutive group segments packed up to CALL
    calls = []  # (band, token_off, n_tokens, [(g, ntiles), ...])
    for b in range(NB):
        cur = None
        for g in range(NG):
            s = int(seg[b, g])
            if s == 0:
                continue
            if cur is None or cur[2] + s > CALL:
                if cur is not None:
                    calls.append(tuple(cur))
                cur = [b, int(off[b, g]), 0, []]
            cur[2] += s
            cur[3].append((g, s // 128))
        if cur is not None:
            calls.append(tuple(cur))

    firstb = np.full(NG, -1, np.int64)
    for g in range(NG):
        for b in range(NB):
            if seg[b, g] > 0:
                firstb[g] = b
                break

    # per-core token arrays: gather idx (int16, band-relative row) and
    # target offset within group (int16, -1 for pad)
    gpacks, tpacks = [], []
    for c in range(NCORES):
        m = core_of == c
        eb, eg = b_all[m], g_all[m]
        etoff, egv = toff_all[m], gval_all[m]
        order = np.lexsort((eg, eb))
        eb, eg, etoff, egv = eb[order], eg[order], etoff[order], egv[order]
        gflat = np.zeros(TOT, np.int16)
        tflat = np.full(TOT, -1, np.int16)
        segkey = eb * NG + eg
        uniq, starts = np.unique(segkey, return_index=True)
        starts = list(starts) + [segkey.size]
        for i in range(len(uniq)):
            b, g = divmod(int(uniq[i]), NG)
            a, e = int(starts[i]), int(starts[i + 1])
            o = int(off[b, g])
            gflat[o:o + e - a] = egv[a:e].astype(np.int16)
            tflat[o:o + e - a] = etoff[a:e].astype(np.int16)
        # gather idx tile layout: token m -> (m % 16, m // 16), replicated
        # to all 128 partitions (the SWDGE tx/rx Q7 cores read different
        # partition groups)
        gpacks.append(np.tile(gflat.reshape(-1, 16).T, (8, 1)))
        # tgt-offset layout matches the gather data layout: token m ->
        # (m % 128, m // 128)
        tpacks.append(np.ascontiguousarray(tflat.reshape(-1, 128).T))

    normb = []
    for c in range(NCORES):
        v = np.zeros((LOC,), np.float32)
        v[:n_loc] = inv_norm[c * n_loc:(c + 1) * n_loc]
        normb.append(np.broadcast_to(v[None, :], (128, LOC)).astype(NP_ACT).copy())

    return dict(
        n_loc=n_loc, CH=CH, LOC=LOC, NR=NR, BAND=BAND, TOT=TOT,
        calls=calls, firstb=firstb, gpacks=gpacks, tpacks=tpacks, normb=normb,
    )


# ----------------------------------------------------------------------------
# Bass kernel build
# ----------------------------------------------------------------------------

def _build_bass(D, FL, OD, CH, LOC, NR, BAND, TOT, calls, firstb):
    assert D == 128
    nc = bacc.Bacc(num_swdge_queues=NQ)

    xT = nc.declare_dram_parameter("xT", [D, LOC], ACT_DT, isOutput=False)
    wmp = nc.declare_dram_parameter("wmp", [D, 3 * D], ACT_DT, isOutput=False)
    bmp = nc.declare_dram_parameter("bmp", [D, 3], F32, isOutput=False)
    nrmb = nc.declare_dram_parameter("nrmb", [128, LOC], ACT_DT, isOutput=False)
    gidx = nc.declare_dram_parameter("gidx", [128, TOT // 16], I16, isOutput=False)
    tg16 = nc.declare_dram_parameter("tg16", [128, TOT // 128], I16, isOutput=False)
    MF = FL // 128
    fc0 = nc.declare_dram_parameter("fc0", [D, FL], ACT_DT, isOutput=False)
    p0 = nc.declare_dram_parameter("p0", [D, FL], ACT_DT, isOutput=False)
    fc1 = nc.declare_dram_parameter("fc1", [128, MF * FL], ACT_DT, isOutput=False)
    p1 = nc.declare_dram_parameter("p1", [D, FL], ACT_DT, isOutput=False)
    ow = nc.declare_dram_parameter("ow", [128, MF * OD], ACT_DT, isOutput=False)
    bz1 = nc.declare_dram_parameter("bz1", [128, MF], F32, isOutput=False)
    bz2 = nc.declare_dram_parameter("bz2", [128, MF], F32, isOutput=False)
    bo = nc.declare_dram_parameter("bo", [128, 1], F32, isOutput=False)
    out_t = nc.declare_dram_parameter("out_t", [OD, LOC], F32, isOutput=True)

    NT = -(-LOC // 512)
    TROWS = -(-NR // 128) * 128 + 128

    with tile.TileContext(nc) as tc:
        with (
            tc.tile_pool(name="dram", bufs=1, space="DRAM") as dram,
            tc.tile_pool(name="big", bufs=1) as big,
            tc.tile_pool(name="wts", bufs=1) as wts,
            tc.tile_pool(name="gb", bufs=3) as gb,
            tc.tile_pool(name="sbb", bufs=3) as sbb,
            tc.tile_pool(name="work", bufs=3) as work,
            tc.tile_pool(name="psmm", bufs=3, space="PSUM") as psmm,
            tc.tile_pool(name="pssc", bufs=3, space="PSUM") as pssc,
            tc.tile_pool(name="pstr", bufs=2, space="PSUM") as pstr,
        ):
            shard = dram.tile([LOC, D], TBL_DT)
            tables = [dram.tile([TROWS, D], TBL_DT, addr_space="Shared",
                                name=f"table{i}") for i in range(3)]

            fmA = big.tile([128, LOC], ACT_DT, tag="fmA")
            fmB = big.tile([128, LOC], ACT_DT, tag="fmB")
            fms = [fmA, fmB]
            nb_s = big.tile([128, LOC], ACT_DT, tag="nb")

            wmm = wts.tile([128, 3 * D], ACT_DT)
            bcol = wts.tile([128, 3], F32)
            ident = wts.tile([128, 128], ACT_DT)
            iota16 = wts.tile([128, 128], I16)
            tg_s = wts.tile([128, TOT // 128], I16)
            gi_s = wts.tile([128, TOT // 16], I16)
            fc0_s = wts.tile([128, FL], ACT_DT)
            p0_s = wts.tile([128, FL], ACT_DT)
            fc1_s = wts.tile([128, MF * FL], ACT_DT)
            p1_s = wts.tile([128, FL], ACT_DT)
            ow_s = wts.tile([128, MF * OD], ACT_DT)
            bz1_s = wts.tile([128, MF], F32)
            bz2_s = wts.tile([128, MF], F32)
            bo_s = wts.tile([128, 1], F32)

            make_identity(nc, ident[:])
            nc.gpsimd.iota(iota16[:], pattern=[[1, 128]], base=0,
                           channel_multiplier=0)

            nc.sync.dma_start(out=fmA[:], in_=xT[:, :])
            nc.sync.dma_start(out=nb_s[:], in_=nrmb[:, :])
            nc.sync.dma_start(out=wmm[:], in_=wmp[:, :])
            nc.sync.dma_start(out=bcol[:], in_=bmp[:, :])
            nc.sync.dma_start(out=tg_s[:], in_=tg16[:, :])
            nc.sync.dma_start(out=gi_s[:], in_=gidx[:, :])
            nc.sync.dma_start(out=fc0_s[:], in_=fc0[:, :])
            nc.sync.dma_start(out=p0_s[:], in_=p0[:, :])
            nc.sync.dma_start(out=fc1_s[:], in_=fc1[:, :])
            nc.sync.dma_start(out=p1_s[:], in_=p1[:, :])
            nc.sync.dma_start(out=ow_s[:], in_=ow[:, :])
            nc.sync.dma_start(out=bz1_s[:], in_=bz1[:, :])
            nc.sync.dma_start(out=bz2_s[:], in_=bz2[:, :])
            nc.sync.dma_start(out=bo_s[:], in_=bo[:, :])

            shard_re = shard[:].rearrange("(p x) d -> p (x d)", p=128)
            qi = 0

            for L in range(_N_LAYERS):
                fm_in = fms[L % 2]
                fm_out = fms[(L + 1) % 2]
                table = tables[L]
                # h = relu(prev @ W + b) * inv_norm  (feat-major, in place)
                for t in range(NT):
                    w = min(512, LOC - t * 512)
                    sl = slice(t * 512, t * 512 + w)
                    ps = psmm.tile([128, 512], F32, tag="mm")
                    nc.tensor.matmul(
                        out=ps[:, :w], lhsT=wmm[:, L * D:(L + 1) * D],
                        rhs=fm_in[:, sl], start=True, stop=True)
                    nc.scalar.activation(
                        out=fm_in[:, sl], in_=ps[:, :w],
                        func=mybir.ActivationFunctionType.Relu,
                        bias=bcol[:, L:L + 1], scale=1.0)
                    nc.vector.tensor_tensor(
                        out=fm_in[:, sl], in0=fm_in[:, sl], in1=nb_s[:, sl],
                        op=mybir.AluOpType.mult)
                # transpose chunks to node-major and publish the shard
                for ch in range(CH):
                    pt = pstr.tile([128, 128], ACT_DT, tag="tr")
                    nc.tensor.transpose(
                        out=pt[:], in_=fm_in[:, ch * 128:(ch + 1) * 128],
                        identity=ident[:])
                    st = work.tile([128, 128], ACT_DT, tag="st")
                    nc.scalar.activation(
                        out=st[:], in_=pt[:],
                        func=mybir.ActivationFunctionType.Copy, scale=1.0)
                    nc.sync.dma_start(
                        out=shard_re[:, ch * D:(ch + 1) * D], in_=st[:])
                if not _SKIP_AG:
                    nc.gpsimd.collective_compute(
                        "AllGather", mybir.AluOpType.bypass,
                        ins=[shard[:]], outs=[table[0:NR, :]],
                        replica_groups=[list(range(NCORES))])
                # edge aggregation: gather source rows; one-hot segment
                # matmuls accumulate Ah into feat-major psum; merge into
                # fm_out = (1+eps)*h + Ah
                merged = set()
                for (b, off, n, runs) in ([] if _SKIP_EDGE else calls):
                    gbuf = gb.tile([128, CALL], TBL_DT, tag="gbuf")
                    if not _SKIP_GATHER:
                        nc.gpsimd.dma_gather(
                            out_ap=gbuf[:, :n].rearrange("p (j e) -> p j e", e=128),
                            in_ap=table[b * BAND:(b + 1) * BAND, :],
                            idxs_ap=gi_s[:, off // 16:(off + n) // 16],
                            num_idxs=n, num_idxs_reg=n, elem_size=128,
                            single_packet=False, queue_num=qi % NQ)
                        qi += 1
                    if _SKIP_SCATTER:
                        continue
                    nt_call = n // 128
                    S = sbb.tile([128, CALL], ACT_DT, tag="S")
                    nc.vector.tensor_tensor(
                        out=S[:, :n].rearrange("p (j t) -> p j t", t=128),
                        in0=iota16[:, :].rearrange("p (o t) -> p o t", o=1)
                            .to_broadcast([128, nt_call, 128]),
                        in1=tg_s[:, off // 128:(off + n) // 128]
                            .rearrange("p (j o) -> p j o", o=1)
                            .to_broadcast([128, nt_call, 128]),
                        op=mybir.AluOpType.is_equal)
                    jbase = 0
                    for (g, ntiles) in runs:
                        ps = pssc.tile([128, 128], F32, tag="sc")
                        for k in range(ntiles):
                            j = jbase + k
                            nc.tensor.matmul(
                                out=ps[:],
                                lhsT=gbuf[:, j * 128:(j + 1) * 128],
                                rhs=S[:, j * 128:(j + 1) * 128],
                                start=(k == 0), stop=(k == ntiles - 1))
                        jbase += ntiles
                        cols = slice(g * 128, (g + 1) * 128)
                        if b == firstb[g]:
                            nc.vector.scalar_tensor_tensor(
                                out=fm_out[:, cols], in0=fm_in[:, cols],
                                scalar=float(1.0 + EPS), in1=ps[:],
                                op0=mybir.AluOpType.mult,
                                op1=mybir.AluOpType.add)
                        else:
                            nc.vector.tensor_tensor(
                                out=fm_out[:, cols], in0=fm_out[:, cols],
                                in1=ps[:], op=mybir.AluOpType.add)
                        merged.add(g)
                # groups with no edges anywhere (or everything skipped)
                for g in range(CH):
                    if g in merged:
                        continue
                    cols = slice(g * 128, (g + 1) * 128)
                    nc.vector.tensor_scalar(
                        out=fm_out[:, cols], in0=fm_in[:, cols],
                        scalar1=float(1.0 + EPS), scalar2=None,
                        op0=mybir.AluOpType.mult)

            fm = fms[_N_LAYERS % 2]

            # ---------------- MLP tail (feat-major) ----------------
            if _SKIP_TAIL:
                for t in range(NT):
                    w = min(512, LOC - t * 512)
                    osb = work.tile([OD, 512], F32, tag="osb")
                    nc.scalar.activation(
                        out=osb[:, :w], in_=fm[:OD, t * 512:t * 512 + w],
                        func=mybir.ActivationFunctionType.Copy, scale=1.0)
                    nc.sync.dma_start(out=out_t[:, t * 512:t * 512 + w],
                                      in_=osb[:, :w])
            zr1 = big.tile([128, MF * 512], ACT_DT, tag="zr1")
            z2 = big.tile([128, MF * 512], ACT_DT, tag="z2")
            for t in range(0 if _SKIP_TAIL else NT):
                w = min(512, LOC - t * 512)
                sl = slice(t * 512, t * 512 + w)
                rlH = work.tile([128, 512], ACT_DT, tag="rlH")
                nc.scalar.activation(
                    out=rlH[:, :w], in_=fm[:, sl],
                    func=mybir.ActivationFunctionType.Relu, scale=1.0)
                for m in range(MF):
                    ps = psmm.tile([128, 512], F32, tag="mm")
                    nc.tensor.matmul(
                        out=ps[:, :w], lhsT=fc0_s[:, m * 128:(m + 1) * 128],
                        rhs=rlH[:, :w], start=True, stop=False)
                    nc.tensor.matmul(
                        out=ps[:, :w], lhsT=p0_s[:, m * 128:(m + 1) * 128],
                        rhs=fm[:, sl], start=False, stop=True)
                    nc.scalar.activation(
                        out=zr1[:, m * 512:m * 512 + w], in_=ps[:, :w],
                        func=mybir.ActivationFunctionType.Relu,
                        bias=bz1_s[:, m:m + 1], scale=1.0)
                for m in range(MF):
                    ps = psmm.tile([128, 512], F32, tag="mm")
                    for k in range(MF):
                        nc.tensor.matmul(
                            out=ps[:, :w],
                            lhsT=fc1_s[:, k * FL + m * 128:k * FL + (m + 1) * 128],
                            rhs=zr1[:, k * 512:k * 512 + w],
                            start=(k == 0), stop=False)
                    nc.tensor.matmul(
                        out=ps[:, :w], lhsT=p1_s[:, m * 128:(m + 1) * 128],
                        rhs=fm[:, sl], start=False, stop=True)
                    nc.vector.tensor_scalar(
                        out=z2[:, m * 512:m * 512 + w], in0=ps[:, :w],
                        scalar1=bz2_s[:, m:m + 1], scalar2=None,
                        op0=mybir.AluOpType.add)
                pso = psmm.tile([128, 512], F32, tag="mm")
                for k in range(MF):
                    nc.tensor.matmul(
                        out=pso[:OD, :w], lhsT=ow_s[:, k * OD:(k + 1) * OD],
                        rhs=z2[:, k * 512:k * 512 + w],
                        start=(k == 0), stop=(k == MF - 1))
                osb = work.tile([OD, 512], F32, tag="osb")
                nc.vector.tensor_scalar(
                    out=osb[:, :w], in0=pso[:OD, :w], scalar1=bo_s[:OD, :],
                    scalar2=None, op0=mybir.AluOpType.add)
                nc.sync.dma_start(out=out_t[:, sl], in_=osb[:, :w])

    nc.compile()
    return nc


# ----------------------------------------------------------------------------
# Entry point
# ----------------------------------------------------------------------------

def kernel(x, edge_index, mpW0, mpb0, mpW1, mpb1, mpW2, mpb2,
           fcW0, fcb0, fcW1, fcb1, pW0, pb0, pW1, pb1, outW, outb,
           _run=None):
    global LAST_RESULTS
    x = np.asarray(x)
    N, D = x.shape
    FL = fcW0.shape[1]
    OD = outW.shape[1]
    MF = FL // 128
    g = _prep_graph(N, edge_index)
    CH, LOC, NR, BAND, TOT = g["CH"], g["LOC"], g["NR"], g["BAND"], g["TOT"]

    s = np.float32(1.0 / np.sqrt(np.float32(D)))
    wmp = np.concatenate([np.asarray(w, np.float32) * s
                          for w in (mpW0, mpW1, mpW2)], axis=1).astype(NP_ACT)
    bmp = np.stack([np.asarray(b, np.float32) * s
                    for b in (mpb0, mpb1, mpb2)], axis=1)

    fc1_pack = np.asarray(fcW1, np.float32).reshape(MF, 128, FL)
    fc1_pack = fc1_pack.transpose(1, 0, 2).reshape(128, MF * FL).astype(NP_ACT)
    ow_pack = np.asarray(outW, np.float32).reshape(MF, 128, OD)
    ow_pack = ow_pack.transpose(1, 0, 2).reshape(128, MF * OD).astype(NP_ACT)
    bz1 = (np.asarray(fcb0, np.float32) + np.asarray(pb0, np.float32)).reshape(MF, 128).T.copy()
    bz2 = (np.asarray(fcb1, np.float32) + np.asarray(pb1, np.float32)).reshape(MF, 128).T.copy()
    bo = np.zeros((128, 1), np.float32)
    bo[:OD, 0] = np.asarray(outb, np.float32)

    nc = _build_bass(D, FL, OD, CH, LOC, NR, BAND, TOT, g["calls"], g["firstb"])

    n_loc = g["n_loc"]
    in_maps = []
    for c in range(NCORES):
        xt = np.zeros((D, LOC), NP_ACT)
        xt[:, :n_loc] = x[c * n_loc:(c + 1) * n_loc].T.astype(NP_ACT)
        in_maps.append(dict(
            xT=xt, wmp=wmp, bmp=bmp, nrmb=g["normb"][c],
            gidx=g["gpacks"][c], tg16=g["tpacks"][c],
            fc0=np.asarray(fcW0, np.float32).astype(NP_ACT),
            p0=np.asarray(pW0, np.float32).astype(NP_ACT),
            fc1=fc1_pack, p1=np.asarray(pW1, np.float32).astype(NP_ACT),
            ow=ow_pack,
            bz1=bz1, bz2=bz2, bo=bo,
        ))

    if _run is None:
        res = run_bass_kernel_spmd(nc, in_maps, list(range(NCORES)), trace=False)
        LAST_RESULTS = res
        outs = [res.results[c]["out_t"] for c in range(NCORES)]
    else:
        outs = _run(nc, in_maps)

    out = np.empty((N, OD), np.float32)
    for c in range(NCORES):
        o = np.asarray(outs[c]).T  # [LOC, OD], row nid
        out[c * n_loc:(c + 1) * n_loc] = o[:n_loc]
    return out


# revision 19
# speedup vs baseline: 51.3954x; 3.1395x over previous
"""Trainium2 Bass kernel for nn_DecoupleModel (GNN message passing, 3 MP layers + MLP tail).

Self-contained: call kernel(**inputs) with the full (unsharded) inputs from
setup_inputs(); returns the full [N, 64] float32 output.

Strategy (8 NeuronCores, node-parallel / graph-parallel):
  - Nodes are sharded by contiguous orig-id range (12500/core, padded to 12544).
  - Per layer: feat-major W matmul (bf16) -> relu(+bias) -> per-node 1/norm
    scale (feat-major, via a host-broadcast column-scale matrix) -> PE
    transpose to node-major -> DMA shard to DRAM -> AllGather the feature
    table into Shared DRAM (halo exchange).
  - Edge aggregation Ah[t] = sum_e h[src(e)] is gather + PE segment-matmul:
    edges are grouped host-side by (source band, target group of 128) and
    padded per segment to a multiple of 128 (sizes shared across cores for
    SPMD). dma_gather (SWDGE) pulls per-edge source rows into SBUF tiles of
    128 edges x 128 feat; a one-hot matrix S (built on the DVE by comparing
    an iota row against each token's target offset) turns the segment-sum
    into psum_fm[feat, tgt] += g_tile^T.T @ S_tile accumulated in f32 PSUM.
    Partials merge into the (feat-major) next-layer activations with
    (1+eps)*h + Ah on the DVE, so no scatter DMA and no transpose back.
  - MLP tail computed feat-major with PSUM-fused injection branches.
  - Host reassembles the final output from the per-core shards.
"""

import os

import numpy as np

import concourse.bass as bass
import concourse.bacc as bacc
import concourse.mybir as mybir
import concourse.tile as tile
from concourse.bass_utils import run_bass_kernel_spmd
from concourse.masks import make_identity

EPS = 2.0 ** 0.5
NCORES = 8
F32 = mybir.dt.float32
BF16 = mybir.dt.bfloat16
I16 = mybir.dt.int16
ACT_DT = BF16                      # activations / weights dtype
NP_ACT = mybir.dt.np(ACT_DT)
TBL_DT = BF16                      # feature table / gather dtype
CALL = int(os.environ.get("GCALL", "4096"))  # max gather tokens per SWDGE call
SP = bool(int(os.environ.get("GSP", "0")))   # single_packet gathers
NQ = int(os.environ.get("NQ", "4"))  # SWDGE queues used (round-robin)

LAST_RESULTS = None

# ablation flags (bench only; all default off)
_SKIP_EDGE = bool(int(os.environ.get("SKIP_EDGE", "0")))
_SKIP_SCATTER = bool(int(os.environ.get("SKIP_SCATTER", "0")))
_SKIP_GATHER = bool(int(os.environ.get("SKIP_GATHER", "0")))
_SKIP_AG = bool(int(os.environ.get("SKIP_AG", "0")))
_SKIP_TAIL = bool(int(os.environ.get("SKIP_TAIL", "0")))
_N_LAYERS = int(os.environ.get("N_LAYERS", "3"))


# ----------------------------------------------------------------------------
# Host-side graph preprocessing
# ----------------------------------------------------------------------------

def _prep_graph(N, edge_index):
    tgt = np.asarray(edge_index[0]).astype(np.int64)
    src = np.asarray(edge_index[1]).astype(np.int64)

    n_loc = N // NCORES
    CH = -(-n_loc // 128)
    LOC = CH * 128
    NR = NCORES * LOC
    BAND = 2 * LOC                 # int16-indexable gather window (<= 32768)
    NB = -(-NR // BAND)
    NG = CH                        # target groups of 128 (psum/merge blocks)

    deg_src = np.bincount(src, minlength=N)
    inv_norm = (1.0 / (1.0 + EPS + deg_src.astype(np.float64))).astype(np.float32)

    # table row of orig node n: core*LOC + (nid%128)*CH + nid//128
    # (so the per-chunk node-major transpose output DMAs contiguously)
    allnid = np.arange(N) % n_loc
    trow = (np.arange(N) // n_loc) * LOC + (allnid % 128) * CH + allnid // 128

    core_of = tgt // n_loc
    tnid = tgt % n_loc
    g_all = tnid // 128
    toff_all = tnid % 128
    erow = trow[src]
    b_all = erow // BAND
    gval_all = erow % BAND

    # shared segment sizes: per (band, group), max over cores, padded to 128
    keys = (core_of * NB + b_all) * NG + g_all
    cnt = np.bincount(keys, minlength=NCORES * NB * NG).reshape(NCORES, NB, NG)
    seg = (-(-cnt.max(axis=0) // 128) * 128).astype(np.int64)  # [NB, NG]

    off = np.zeros((NB, NG), np.int64)
    tot = 0
    for b in range(NB):
        for g in range(NG):
            off[b, g] = tot
            tot += int(seg[b, g])
    TOT = tot

    # call list: per band, consecutive group segments packed up to CALL
    calls = []  # (band, token_off, n_tokens, [(g, ntiles), ...])
    for b in range(NB):
        cur = None
        for g in range(NG):
            s = int(seg[b, g])
            if s == 0:
                continue
            if cur is None or cur[2] + s > CALL:
                if cur is not None:
                    calls.append(tuple(cur))
                cur = [b, int(off[b, g]), 0, []]
            cur[2] += s
            cur[3].append((g, s // 128))
        if cur is not None:
            calls.append(tuple(cur))

    firstb = np.full(NG, -1, np.int64)
    for g in range(NG):
        for b in range(NB):
            if seg[b, g] > 0:
                firstb[g] = b
                break

    # per-core token arrays: gather idx (int16, band-relative row) and
    # target offset within group (int16, -1 for pad)
    gpacks, tpacks = [], []
    for c in range(NCORES):
        m = core_of == c
        eb, eg = b_all[m], g_all[m]
        etoff, egv = toff_all[m], gval_all[m]
        order = np.lexsort((eg, eb))
        eb, eg, etoff, egv = eb[order], eg[order], etoff[order], egv[order]
        gflat = np.zeros(TOT, np.int16)
        tflat = np.full(TOT, -1, np.int16)
        segkey = eb * NG + eg
        uniq, starts = np.unique(segkey, return_index=True)
        starts = list(starts) + [segkey.size]
        for i in range(len(uniq)):
            b, g = divmod(int(uniq[i]), NG)
            a, e = int(starts[i]), int(starts[i + 1])
            o = int(off[b, g])
            gflat[o:o + e - a] = egv[a:e].astype(np.int16)
            tflat[o:o + e - a] = etoff[a:e].astype(np.int16)
        # gather idx tile layout: token m -> (m % 16, m // 16), replicated
        # to all 128 partitions (the SWDGE tx/rx Q7 cores read different
        # partition groups)
        gpacks.append(np.tile(gflat.reshape(-1, 16).T, (8, 1)))
        # tgt-offset layout matches the gather data layout: token m ->
        # (m % 128, m // 128)
        tpacks.append(np.ascontiguousarray(tflat.reshape(-1, 128).T))

    normb = []
    for c in range(NCORES):
        v = np.zeros((LOC,), np.float32)
        v[:n_loc] = inv_norm[c * n_loc:(c + 1) * n_loc]
        normb.append(np.broadcast_to(v[None, :], (128, LOC)).astype(NP_ACT).copy())

    return dict(
        n_loc=n_loc, CH=CH, LOC=LOC, NR=NR, BAND=BAND, TOT=TOT,
        calls=calls, firstb=firstb, gpacks=gpacks, tpacks=tpacks, normb=normb,
    )


# ----------------------------------------------------------------------------
# Bass kernel build
# ----------------------------------------------------------------------------

def _build_bass(D, FL, OD, CH, LOC, NR, BAND, TOT, calls, firstb):
    assert D == 128
    nc = bacc.Bacc(num_swdge_queues=NQ)

    xT = nc.declare_dram_parameter("xT", [D, LOC], ACT_DT, isOutput=False)
    wmp = nc.declare_dram_parameter("wmp", [D, 3 * D], ACT_DT, isOutput=False)
    bmp = nc.declare_dram_parameter("bmp", [D, 3], F32, isOutput=False)
    nrmb = nc.declare_dram_parameter("nrmb", [128, LOC], ACT_DT, isOutput=False)
    gidx = nc.declare_dram_parameter("gidx", [128, TOT // 16], I16, isOutput=False)
    tg16 = nc.declare_dram_parameter("tg16", [128, TOT // 128], I16, isOutput=False)
    MF = FL // 128
    fc0 = nc.declare_dram_parameter("fc0", [D, FL], ACT_DT, isOutput=False)
    p0 = nc.declare_dram_parameter("p0", [D, FL], ACT_DT, isOutput=False)
    fc1 = nc.declare_dram_parameter("fc1", [128, MF * FL], ACT_DT, isOutput=False)
    p1 = nc.declare_dram_parameter("p1", [D, FL], ACT_DT, isOutput=False)
    ow = nc.declare_dram_parameter("ow", [128, MF * OD], ACT_DT, isOutput=False)
    bz1 = nc.declare_dram_parameter("bz1", [128, MF], F32, isOutput=False)
    bz2 = nc.declare_dram_parameter("bz2", [128, MF], F32, isOutput=False)
    bo = nc.declare_dram_parameter("bo", [128, 1], F32, isOutput=False)
    out_t = nc.declare_dram_parameter("out_t", [OD, LOC], F32, isOutput=True)

    NT = -(-LOC // 512)
    TROWS = -(-NR // 128) * 128 + 128

    with tile.TileContext(nc) as tc:
        with (
            tc.tile_pool(name="dram", bufs=1, space="DRAM") as dram,
            tc.tile_pool(name="big", bufs=1) as big,
            tc.tile_pool(name="wts", bufs=1) as wts,
            tc.tile_pool(name="gb", bufs=3) as gb,
            tc.tile_pool(name="sbb", bufs=3) as sbb,
            tc.tile_pool(name="work", bufs=3) as work,
            tc.tile_pool(name="psmm", bufs=3, space="PSUM") as psmm,
            tc.tile_pool(name="pssc", bufs=3, space="PSUM") as pssc,
            tc.tile_pool(name="pstr", bufs=2, space="PSUM") as pstr,
        ):
            shard = dram.tile([LOC, D], TBL_DT)
            tables = [dram.tile([TROWS, D], TBL_DT, addr_space="Shared",
                                name=f"table{i}") for i in range(3)]

            fmA = big.tile([128, LOC], ACT_DT, tag="fmA")
            fmB = big.tile([128, LOC], ACT_DT, tag="fmB")
            fms = [fmA, fmB]
            nb_s = big.tile([128, LOC], ACT_DT, tag="nb")

            wmm = wts.tile([128, 3 * D], ACT_DT)
            bcol = wts.tile([128, 3], F32)
            ident = wts.tile([128, 128], ACT_DT)
            iota16 = wts.tile([128, 128], I16)
            tg_s = wts.tile([128, TOT // 128], I16)
            gi_s = wts.tile([128, TOT // 16], I16)
            fc0_s = wts.tile([128, FL], ACT_DT)
            p0_s = wts.tile([128, FL], ACT_DT)
            fc1_s = wts.tile([128, MF * FL], ACT_DT)
            p1_s = wts.tile([128, FL], ACT_DT)
            ow_s = wts.tile([128, MF * OD], ACT_DT)
            bz1_s = wts.tile([128, MF], F32)
            bz2_s = wts.tile([128, MF], F32)
            bo_s = wts.tile([128, 1], F32)

            make_identity(nc, ident[:])
            nc.gpsimd.iota(iota16[:], pattern=[[1, 128]], base=0,
                           channel_multiplier=0)

            nc.sync.dma_start(out=fmA[:], in_=xT[:, :])
            nc.sync.dma_start(out=nb_s[:], in_=nrmb[:, :])
            nc.sync.dma_start(out=wmm[:], in_=wmp[:, :])
            nc.sync.dma_start(out=bcol[:], in_=bmp[:, :])
            nc.sync.dma_start(out=tg_s[:], in_=tg16[:, :])
            nc.sync.dma_start(out=gi_s[:], in_=gidx[:, :])
            nc.sync.dma_start(out=fc0_s[:], in_=fc0[:, :])
            nc.sync.dma_start(out=p0_s[:], in_=p0[:, :])
            nc.sync.dma_start(out=fc1_s[:], in_=fc1[:, :])
            nc.sync.dma_start(out=p1_s[:], in_=p1[:, :])
            nc.sync.dma_start(out=ow_s[:], in_=ow[:, :])
            nc.sync.dma_start(out=bz1_s[:], in_=bz1[:, :])
            nc.sync.dma_start(out=bz2_s[:], in_=bz2[:, :])
            nc.sync.dma_start(out=bo_s[:], in_=bo[:, :])

            shard_re = shard[:].rearrange("(p x) d -> p (x d)", p=128)
            qi = 0

            for L in range(_N_LAYERS):
                fm_in = fms[L % 2]
                fm_out = fms[(L + 1) % 2]
                table = tables[L]
                # h = relu(prev @ W + b) * inv_norm  (feat-major, in place)
                for t in range(NT):
                    w = min(512, LOC - t * 512)
                    sl = slice(t * 512, t * 512 + w)
                    ps = psmm.tile([128, 512], F32, tag="mm")
                    nc.tensor.matmul(
                        out=ps[:, :w], lhsT=wmm[:, L * D:(L + 1) * D],
                        rhs=fm_in[:, sl], start=True, stop=True)
                    nc.scalar.activation(
                        out=fm_in[:, sl], in_=ps[:, :w],
                        func=mybir.ActivationFunctionType.Relu,
                        bias=bcol[:, L:L + 1], scale=1.0)
                    nc.vector.tensor_tensor(
                        out=fm_in[:, sl], in0=fm_in[:, sl], in1=nb_s[:, sl],
                        op=mybir.AluOpType.mult)
                # transpose chunks to node-major and publish the shard
                for ch in range(CH):
                    pt = pstr.tile([128, 128], ACT_DT, tag="tr")
                    nc.tensor.transpose(
                        out=pt[:], in_=fm_in[:, ch * 128:(ch + 1) * 128],
                        identity=ident[:])
                    st = work.tile([128, 128], ACT_DT, tag="st")
                    nc.scalar.activation(
                        out=st[:], in_=pt[:],
                        func=mybir.ActivationFunctionType.Copy, scale=1.0)
                    nc.sync.dma_start(
                        out=shard_re[:, ch * D:(ch + 1) * D], in_=st[:])
                if not _SKIP_AG:
                    nc.gpsimd.collective_compute(
                        "AllGather", mybir.AluOpType.bypass,
                        ins=[shard[:]], outs=[table[0:NR, :]],
                        replica_groups=[list(range(NCORES))])
                # edge aggregation: gather source rows; one-hot segment
                # matmuls accumulate Ah into feat-major psum; merge into
                # fm_out = (1+eps)*h + Ah
                merged = set()
                for (b, off, n, runs) in ([] if _SKIP_EDGE else calls):
                    gbuf = gb.tile([128, CALL], TBL_DT, tag="gbuf")
                    if not _SKIP_GATHER:
                        nc.gpsimd.dma_gather(
                            out_ap=gbuf[:, :n].rearrange("p (j e) -> p j e", e=128),
                            in_ap=table[b * BAND:(b + 1) * BAND, :],
                            idxs_ap=gi_s[:, off // 16:(off + n) // 16],
                            num_idxs=n, num_idxs_reg=n, elem_size=128,
                            single_packet=SP, queue_num=qi % NQ)
                        qi += 1
                    if _SKIP_SCATTER:
                        continue
                    nt_call = n // 128
                    S = sbb.tile([128, CALL], ACT_DT, tag="S")
                    nc.vector.tensor_tensor(
                        out=S[:, :n].rearrange("p (j t) -> p j t", t=128),
                        in0=iota16[:, :].rearrange("p (o t) -> p o t", o=1)
                            .to_broadcast([128, nt_call, 128]),
                        in1=tg_s[:, off // 128:(off + n) // 128]
                            .rearrange("p (j o) -> p j o", o=1)
                            .to_broadcast([128, nt_call, 128]),
                        op=mybir.AluOpType.is_equal)
                    jbase = 0
                    for (g, ntiles) in runs:
                        ps = pssc.tile([128, 128], F32, tag="sc")
                        for k in range(ntiles):
                            j = jbase + k
                            nc.tensor.matmul(
                                out=ps[:],
                                lhsT=gbuf[:, j * 128:(j + 1) * 128],
                                rhs=S[:, j * 128:(j + 1) * 128],
                                start=(k == 0), stop=(k == ntiles - 1))
                        jbase += ntiles
                        cols = slice(g * 128, (g + 1) * 128)
                        if b == firstb[g]:
                            nc.vector.scalar_tensor_tensor(
                                out=fm_out[:, cols], in0=fm_in[:, cols],
                                scalar=float(1.0 + EPS), in1=ps[:],
                                op0=mybir.AluOpType.mult,
                                op1=mybir.AluOpType.add)
                        else:
                            nc.vector.tensor_tensor(
                                out=fm_out[:, cols], in0=fm_out[:, cols],
                                in1=ps[:], op=mybir.AluOpType.add)
                        merged.add(g)
                # groups with no edges anywhere (or everything skipped)
                for g in range(CH):
                    if g in merged:
                        continue
                    cols = slice(g * 128, (g + 1) * 128)
                    nc.vector.tensor_scalar(
                        out=fm_out[:, cols], in0=fm_in[:, cols],
                        scalar1=float(1.0 + EPS), scalar2=None,
                        op0=mybir.AluOpType.mult)

            fm = fms[_N_LAYERS % 2]

            # ---------------- MLP tail (feat-major) ----------------
            if _SKIP_TAIL:
                for t in range(NT):
                    w = min(512, LOC - t * 512)
                    osb = work.tile([OD, 512], F32, tag="osb")
                    nc.scalar.activation(
                        out=osb[:, :w], in_=fm[:OD, t * 512:t * 512 + w],
                        func=mybir.ActivationFunctionType.Copy, scale=1.0)
                    nc.sync.dma_start(out=out_t[:, t * 512:t * 512 + w],
                                      in_=osb[:, :w])
            zr1 = big.tile([128, MF * 512], ACT_DT, tag="zr1")
            z2 = big.tile([128, MF * 512], ACT_DT, tag="z2")
            for t in range(0 if _SKIP_TAIL else NT):
                w = min(512, LOC - t * 512)
                sl = slice(t * 512, t * 512 + w)
                rlH = work.tile([128, 512], ACT_DT, tag="rlH")
                nc.scalar.activation(
                    out=rlH[:, :w], in_=fm[:, sl],
                    func=mybir.ActivationFunctionType.Relu, scale=1.0)
                for m in range(MF):
                    ps = psmm.tile([128, 512], F32, tag="mm")
                    nc.tensor.matmul(
                        out=ps[:, :w], lhsT=fc0_s[:, m * 128:(m + 1) * 128],
                        rhs=rlH[:, :w], start=True, stop=False)
                    nc.tensor.matmul(
                        out=ps[:, :w], lhsT=p0_s[:, m * 128:(m + 1) * 128],
                        rhs=fm[:, sl], start=False, stop=True)
                    nc.scalar.activation(
                        out=zr1[:, m * 512:m * 512 + w], in_=ps[:, :w],
                        func=mybir.ActivationFunctionType.Relu,
                        bias=bz1_s[:, m:m + 1], scale=1.0)
                for m in range(MF):
                    ps = psmm.tile([128, 512], F32, tag="mm")
                    for k in range(MF):
                        nc.tensor.matmul(
                            out=ps[:, :w],
                            lhsT=fc1_s[:, k * FL + m * 128:k * FL + (m + 1) * 128],
                            rhs=zr1[:, k * 512:k * 512 + w],
                            start=(k == 0), stop=False)
                    nc.tensor.matmul(
                        out=ps[:, :w], lhsT=p1_s[:, m * 128:(m + 1) * 128],
                        rhs=fm[:, sl], start=False, stop=True)
                    nc.vector.tensor_scalar(
                        out=z2[:, m * 512:m * 512 + w], in0=ps[:, :w],
                        scalar1=bz2_s[:, m:m + 1], scalar2=None,
                        op0=mybir.AluOpType.add)
                pso = psmm.tile([128, 512], F32, tag="mm")
                for k in range(MF):
                    nc.tensor.matmul(
                        out=pso[:OD, :w], lhsT=ow_s[:, k * OD:(k + 1) * OD],
                        rhs=z2[:, k * 512:k * 512 + w],
                        start=(k == 0), stop=(k == MF - 1))
                osb = work.tile([OD, 512], F32, tag="osb")
                nc.vector.tensor_scalar(
                    out=osb[:, :w], in0=pso[:OD, :w], scalar1=bo_s[:OD, :],
                    scalar2=None, op0=mybir.AluOpType.add)
                nc.sync.dma_start(out=out_t[:, sl], in_=osb[:, :w])

    nc.compile()
    return nc


# ----------------------------------------------------------------------------
# Entry point
# ----------------------------------------------------------------------------

def kernel(x, edge_index, mpW0, mpb0, mpW1, mpb1, mpW2, mpb2,
           fcW0, fcb0, fcW1, fcb1, pW0, pb0, pW1, pb1, outW, outb,
           _run=None):
    global LAST_RESULTS
    x = np.asarray(x)
    N, D = x.shape
    FL = fcW0.shape[1]
    OD = outW.shape[1]
    MF = FL // 128
    g = _prep_graph(N, edge_index)
    CH, LOC, NR, BAND, TOT = g["CH"], g["LOC"], g["NR"], g["BAND"], g["TOT"]

    s = np.float32(1.0 / np.sqrt(np.float32(D)))
    wmp = np.concatenate([np.asarray(w, np.float32) * s
                          for w in (mpW0, mpW1, mpW2)], axis=1).astype(NP_ACT)
    bmp = np.stack([np.asarray(b, np.float32) * s
                    for b in (mpb0, mpb1, mpb2)], axis=1)

    fc1_pack = np.asarray(fcW1, np.float32).reshape(MF, 128, FL)
    fc1_pack = fc1_pack.transpose(1, 0, 2).reshape(128, MF * FL).astype(NP_ACT)
    ow_pack = np.asarray(outW, np.float32).reshape(MF, 128, OD)
    ow_pack = ow_pack.transpose(1, 0, 2).reshape(128, MF * OD).astype(NP_ACT)
    bz1 = (np.asarray(fcb0, np.float32) + np.asarray(pb0, np.float32)).reshape(MF, 128).T.copy()
    bz2 = (np.asarray(fcb1, np.float32) + np.asarray(pb1, np.float32)).reshape(MF, 128).T.copy()
    bo = np.zeros((128, 1), np.float32)
    bo[:OD, 0] = np.asarray(outb, np.float32)

    nc = _build_bass(D, FL, OD, CH, LOC, NR, BAND, TOT, g["calls"], g["firstb"])

    n_loc = g["n_loc"]
    in_maps = []
    for c in range(NCORES):
        xt = np.zeros((D, LOC), NP_ACT)
        xt[:, :n_loc] = x[c * n_loc:(c + 1) * n_loc].T.astype(NP_ACT)
        in_maps.append(dict(
            xT=xt, wmp=wmp, bmp=bmp, nrmb=g["normb"][c],
            gidx=g["gpacks"][c], tg16=g["tpacks"][c],
            fc0=np.asarray(fcW0, np.float32).astype(NP_ACT),
            p0=np.asarray(pW0, np.float32).astype(NP_ACT),
            fc1=fc1_pack, p1=np.asarray(pW1, np.float32).astype(NP_ACT),
            ow=ow_pack,
            bz1=bz1, bz2=bz2, bo=bo,
        ))

    if _run is None:
        res = run_bass_kernel_spmd(nc, in_maps, list(range(NCORES)), trace=False)
        LAST_RESULTS = res
        outs = [res.results[c]["out_t"] for c in range(NCORES)]
    else:
        outs = _run(nc, in_maps)

    out = np.empty((N, OD), np.float32)
    for c in range(NCORES):
        o = np.asarray(outs[c]).T  # [LOC, OD], row nid
        out[c * n_loc:(c + 1) * n_loc] = o[:n_loc]
    return out
